# revision 20
# baseline (speedup 1.0000x reference)
"""Cross-attention kernel for 8 trn2 NeuronCores.

Problem: B=2, Lq=Lk=2048, D=1024, H=16, dh=64.
  q/k/v = Linear(x); q,k L2-normalized per head; S = q@k.T * 1/8;
  key-pad mask -> -1e9; softmax; mask-aware renorm; eps-smooth toward
  uniform-over-valid; out = attn@v merged -> out_proj.

Sharding: core c handles batch b=c//4, heads [4*(c%4), 4*(c%4)+4)
(two head pairs of 2 heads). Each core computes a partial output
projection over its 256 head dims; the host sums the 4 partials per
batch and adds the output bias.

Key optimizations over the v1 baseline:
  - Host-side key compaction: ~half the keys are padding; only valid
    keys (padded to a multiple of 128) are shipped/projected/attended.
    Masked keys contribute exactly 0 to P, rowsum and AV, so dropping
    them is mathematically identical. The Bass program is built per
    runtime KT (cached).
  - Rowsum for free: v carries a 65th column = 1/0.9, so the AV matmul
    output column 64 is rowsum(P)/0.9 and the old per-kt rowsum matmuls
    disappear. AV runs in [q, d] orientation (lhsT = P tile), so the
    renorm divide + eps smoothing collapse into one scalar_tensor_tensor
    with a per-partition 0.9/rs scalar; PE transposes restore [d, q]
    for the output projection.
  - Out-projection interleaved per q-chunk (no serial tail); the
    0.1*vmean eps-smoothing term rides in the stt; host adds bo.
  - Few large DMAs into resident SBUF x tiles instead of 192 small ones.
"""

import ml_dtypes
import numpy as np

import concourse.bass as bass
from concourse import bacc
from concourse import masks
import concourse.mybir as mybir
import concourse.tile as tile
from concourse.bass_utils import run_bass_kernel_spmd

F32 = mybir.dt.float32
BF16 = mybir.dt.bfloat16
FP8 = mybir.dt.float8e4
AF = mybir.ActivationFunctionType
ALU = mybir.AluOpType
DR = mybir.MatmulPerfMode.DoubleRow
W8SCALE = 16.0  # fp8 weight prescale; cancelled exactly by the L2 norm

B, L, D = 2, 2048, 1024
H, DH = 16, 64
HEADS_PER_CORE = 4          # -> 256 dims per core, 2 head pairs
HPC = HEADS_PER_CORE * DH   # 256
SCALE = 0.125               # 1/sqrt(64) / ATTN_TEMP
EPS_SMOOTH = 0.1
INV09 = 1.0 / (1.0 - EPS_SMOOTH)
MASK_BIAS = -30000.0
N_CORES = 8
QC = L // 512               # 4 q chunks
NCH = D // 128              # 8 contraction chunks for projections


def _build_nc(KT):
    LK = KT * 128
    # k-token chunks of <=512 for the k projection
    kchunks = []
    o = 0
    while o < LK:
        sz = min(512, LK - o)
        kchunks.append((o, sz))
        o += sz

    nc = bacc.Bacc(None)

    # q/k side is fp8 (DoubleRow d_in-pair interleaved); v stays bf16
    xq8 = nc.dram_tensor("xq8", [128, NCH // 2, 2, L], FP8,
                         kind="ExternalInput")
    xk8 = nc.dram_tensor("xk8", [128, NCH // 2, 2, LK], FP8,
                         kind="ExternalInput")
    xvT = nc.dram_tensor("xvT", [D, LK], BF16, kind="ExternalInput")
    wq8 = nc.dram_tensor("wq8", [128, NCH // 2, 2, HPC], FP8,
                         kind="ExternalInput")
    wk8 = nc.dram_tensor("wk8", [128, NCH // 2, 2, HPC], FP8,
                         kind="ExternalInput")
    wv_t = nc.dram_tensor("wv_t", [D, HPC], BF16, kind="ExternalInput")
    wo_t = nc.dram_tensor("wo_t", [HPC, D], BF16, kind="ExternalInput")
    bq = nc.dram_tensor("bq", [2, 1, 128], BF16, kind="ExternalInput")
    bk = nc.dram_tensor("bk", [2, 1, 128], BF16, kind="ExternalInput")
    bv = nc.dram_tensor("bv", [1, HPC], BF16, kind="ExternalInput")
    mbias = nc.dram_tensor("mbias", [128, KT], F32, kind="ExternalInput")
    vmb = nc.dram_tensor("vmb", [128, HEADS_PER_CORE, DH], F32,
                         kind="ExternalInput")
    partial = nc.dram_tensor("partial", [L, D], F32, kind="ExternalOutput")

    with tile.TileContext(nc) as tc:
        with (
            tc.tile_pool(name="consts", bufs=1) as consts,
            tc.tile_pool(name="wpool", bufs=1) as wpool,
            tc.tile_pool(name="xres", bufs=1) as xres,
            tc.tile_pool(name="persist", bufs=1) as persist,
            tc.tile_pool(name="l2pool", bufs=4) as l2pool,
            tc.tile_pool(name="ppool", bufs=3) as ppool,
            tc.tile_pool(name="normpool", bufs=4) as normpool,
            tc.tile_pool(name="ostpool", bufs=4) as ostpool,
        ):
            # ---- constants ----
            ones_row = consts.tile([1, 512], BF16, tag="ones_row")
            nc.vector.memset(ones_row, 1.0)
            blockdiag = consts.tile([128, 128], BF16, tag="blockdiag")
            nc.vector.memset(blockdiag, 0.0)
            nc.vector.memset(blockdiag[0:64, 0:64], 1.0)
            nc.vector.memset(blockdiag[64:128, 64:128], 1.0)
            ident = consts.tile([128, 128], BF16, tag="ident")
            masks.make_identity(nc, ident[:, :])
            mbias_sb = consts.tile([128, KT], F32, tag="mbias")
            nc.sync.dma_start(out=mbias_sb, in_=mbias[:, :])
            vmb_sb = consts.tile([128, HEADS_PER_CORE, DH], F32, tag="vmb")
            nc.sync.dma_start(out=vmb_sb, in_=vmb[:, :, :])
            bias_sb = {}
            for name, hnd in (("q", bq), ("k", bk)):
                for hp in range(2):
                    t = consts.tile([1, 128], BF16, tag=f"b{name}{hp}")
                    nc.sync.dma_start(out=t, in_=hnd[hp])
                    bias_sb[(name, hp)] = t
            bv_sb = consts.tile([1, HPC], BF16, tag="bv")
            nc.sync.dma_start(out=bv_sb, in_=bv[:, :])

            # ---- weights ----
            w8_sb = {}
            for name, hnd in (("q", wq8), ("k", wk8)):
                t = wpool.tile([128, NCH // 2, 2, HPC], FP8, tag=f"w8{name}")
                nc.sync.dma_start(out=t, in_=hnd[:, :, :, :])
                w8_sb[name] = t
            wv_sb = wpool.tile([128, NCH, HPC], BF16, tag="wv")
            nc.sync.dma_start(
                out=wv_sb, in_=wv_t.rearrange("(c p) m -> p c m", p=128)
            )
            wo_sb = wpool.tile([128, 2, D], BF16, tag="wo")
            nc.sync.dma_start(
                out=wo_sb, in_=wo_t.rearrange("(h p) m -> p h m", p=128)
            )

            # ---- resident x (big chunked DMAs; k first — it gates attention)
            xq_sb = xres.tile([128, NCH // 2, 2, L], FP8, tag="xq")
            xk_sb = xres.tile([128, NCH // 2, 2, LK], FP8, tag="xk")
            xv_sb = xres.tile([128, NCH, LK], BF16, tag="xv")
            for cc in range(NCH // 2):
                nc.sync.dma_start(
                    out=xk_sb[:, cc, :, :], in_=xk8[:, cc, :, :]
                )
            for c in range(NCH):
                nc.sync.dma_start(
                    out=xv_sb[:, c, :], in_=xvT[c * 128:(c + 1) * 128, :]
                )
            for cc in range(NCH // 2):
                nc.sync.dma_start(
                    out=xq_sb[:, cc, :, :], in_=xq8[:, cc, :, :]
                )

            # ---- persistent activations ----
            # q̂/k̂ in fp8: flat [dims, tokens] per head pair, plus the
            # DoubleRow-interleaved copy [32, head, dim-half, tokens]
            qT8 = persist.tile([128, 2, L], FP8, tag="qT8")
            kT8 = persist.tile([128, 2, LK], FP8, tag="kT8")
            qTi8 = persist.tile([32, HEADS_PER_CORE, 2, L], FP8, tag="qTi8")
            kTi8 = persist.tile([32, HEADS_PER_CORE, 2, LK], FP8, tag="kTi8")
            # v with a 65th column of 1/0.9 per head -> AV col 64 = rs/0.9
            v65 = persist.tile([128, KT, HEADS_PER_CORE, DH + 1], BF16,
                               tag="v65")
            nc.vector.memset(v65[:, :, :, DH:DH + 1], INV09)

            # ---- projections ----
            with (
                tc.tile_pool(name="ps_proj", bufs=4, space="PSUM") as ps_proj,
                tc.tile_pool(name="ps_n2", bufs=2, space="PSUM") as ps_n2,
                tc.tile_pool(name="ps_v", bufs=2, space="PSUM") as ps_v,
            ):
                # q/k: psum [dout 128, tokens] accumulated over d_in chunk
                # pairs (fp8 DoubleRow; weights prescaled by 16, cancelled by
                # the L2 norm), then per-head L2 normalization -> fp8 + the
                # interleave remap DMAs for the fp8 S matmuls.
                # k first, then v, then q: attention consumes k/v first.
                def proj_qk(name, xsb, dst, di8, ntok, chunks):
                    for t0, tsz in chunks:
                        psums = [
                            ps_proj.tile([128, 512], F32, tag="proj",
                                         name=f"proj{i}")
                            for i in range(2)
                        ]
                        for cc in range(NCH // 2):
                            for hp in range(2):
                                nc.tensor.matmul(
                                    psums[hp][:, 0:tsz],
                                    lhsT=w8_sb[name][:, cc, :,
                                                     hp * 128:(hp + 1) * 128],
                                    rhs=xsb[:, cc, :, t0:t0 + tsz],
                                    start=(cc == 0),
                                    stop=False,
                                    perf_mode=DR,
                                )
                        for hp in range(2):
                            # + bias*16 (broadcast along tokens, K=1 matmul)
                            nc.tensor.matmul(
                                psums[hp][:, 0:tsz],
                                lhsT=bias_sb[(name, hp)],
                                rhs=ones_row[:, 0:tsz],
                                start=False,
                                stop=True,
                            )
                            # L2 norm over each head's 64 dims
                            sq = l2pool.tile([128, 512], BF16, tag="sq")
                            nc.scalar.square(sq[:, 0:tsz], psums[hp][:, 0:tsz])
                            n2 = ps_n2.tile([128, 512], F32, tag="n2")
                            nc.tensor.matmul(
                                n2[:, 0:tsz], lhsT=blockdiag,
                                rhs=sq[:, 0:tsz], start=True, stop=True,
                            )
                            nlen = l2pool.tile([128, 512], F32, tag="nlen")
                            nc.scalar.activation(
                                nlen[:, 0:tsz], n2[:, 0:tsz], AF.Sqrt
                            )
                            rnorm = l2pool.tile([128, 512], F32, tag="rnorm")
                            nc.vector.reciprocal_approx_fast(
                                rnorm[:, 0:tsz], nlen[:, 0:tsz]
                            )
                            nc.vector.tensor_mul(
                                dst[:, hp, t0:t0 + tsz],
                                psums[hp][:, 0:tsz], rnorm[:, 0:tsz],
                            )
                    # interleave remap: [h*64+j*32+r, hp, t] -> [r, g, j, t]
                    for g in range(HEADS_PER_CORE):
                        hp, h = g // 2, g % 2
                        for j in range(2):
                            p0 = h * 64 + j * 32
                            nc.sync.dma_start(
                                out=di8[:, g, j, 0:ntok],
                                in_=dst[p0:p0 + 32, hp, 0:ntok],
                            )

                proj_qk("k", xk_sb, kT8, kTi8, LK, kchunks)

                # v: v[t, dout] = sum_c xT[c][:, t].T @ w[c]
                for tt in range(KT):
                    vp = ps_v.tile([128, HPC], F32, tag="vproj")
                    for c in range(NCH):
                        nc.tensor.matmul(
                            vp,
                            lhsT=xv_sb[:, c, tt * 128:(tt + 1) * 128],
                            rhs=wv_sb[:, c, :],
                            start=(c == 0), stop=False,
                        )
                    nc.tensor.matmul(
                        vp, lhsT=ones_row[:, 0:128], rhs=bv_sb,
                        start=False, stop=True,
                    )
                    nc.vector.tensor_copy(v65[:, tt, :, 0:DH], vp)

                proj_qk("q", xq_sb, qT8, qTi8, L,
                        [(i * 512, 512) for i in range(QC)])

            # ---- attention + interleaved out-projection ----
            with (
                tc.tile_pool(name="ps_S", bufs=2, space="PSUM") as ps_S,
                tc.tile_pool(name="ps_O", bufs=1, space="PSUM") as ps_O,
                tc.tile_pool(name="ps_tr", bufs=1, space="PSUM") as ps_tr,
                tc.tile_pool(name="ps_out", bufs=1, space="PSUM") as ps_out,
                tc.tile_pool(name="ofpool", bufs=2) as ofpool,
            ):
                for qc in range(QC):
                    qsl = slice(qc * 512, (qc + 1) * 512)
                    ofin = [
                        ofpool.tile([128, 512], BF16, tag=f"ofin{hp}",
                                    name=f"ofin{hp}")
                        for hp in range(2)
                    ]
                    for hp in range(2):
                        o_ps = [
                            ps_O.tile([128, 4, DH + 1], F32, tag=f"o{h}",
                                      name=f"o{h}")
                            for h in range(2)
                        ]
                        # 8 accumulation regions share these banks at
                        # different free offsets; a start=True reset would
                        # wipe sibling regions, so zero once + accumulate.
                        for h in range(2):
                            nc.vector.memset(o_ps[h], 0.0)
                        for kt in range(KT):
                            s_ps = ps_S.tile([128, 1024], F32, tag="s")
                            ksl = slice(kt * 128, (kt + 1) * 128)
                            # S_T = k̂.T q̂ per head, fp8 DoubleRow (two
                            # 32-dim halves of the 64-dim contraction)
                            for h in range(2):
                                g = hp * 2 + h
                                nc.tensor.matmul(
                                    s_ps[:, h * 512:(h + 1) * 512],
                                    lhsT=kTi8[:, g, :, ksl],
                                    rhs=qTi8[:, g, :, qsl],
                                    start=True, stop=True,
                                    perf_mode=DR,
                                )
                            # P = exp(SCALE*S + pad_bias); padded keys -> 0
                            p_sb = ppool.tile([128, 1024], BF16, tag="p")
                            nc.scalar.activation(
                                p_sb, s_ps, AF.Exp,
                                bias=mbias_sb[:, kt:kt + 1], scale=SCALE,
                            )
                            # O[q, d(+rs)] += P_tile.T @ [v | 1/0.9]
                            for h in range(2):
                                for qi in range(4):
                                    nc.tensor.matmul(
                                        o_ps[h][:, qi, :],
                                        lhsT=p_sb[:, h * 512 + qi * 128:
                                                  h * 512 + (qi + 1) * 128],
                                        rhs=v65[:, kt, hp * 2 + h, :],
                                        start=False, stop=(kt == KT - 1),
                                        skip_group_check=True,
                                    )
                        # renorm + eps smoothing into a head-paired [q, d]
                        # tile, then one [128,128] transpose per q subtile
                        oT = normpool.tile([128, 4, 128], BF16, tag="oT")
                        for h in range(2):
                            rr = normpool.tile([128, 4], F32, tag="rr")
                            nc.vector.reciprocal_approx_fast(
                                rr, o_ps[h][:, :, DH:DH + 1]
                            )
                            for qi in range(4):
                                # (o * 0.9/rs) + 0.1*vmean
                                nc.vector.scalar_tensor_tensor(
                                    out=oT[:, qi, h * DH:(h + 1) * DH],
                                    in0=o_ps[h][:, qi, 0:DH],
                                    scalar=rr[:, qi:qi + 1],
                                    in1=vmb_sb[:, hp * 2 + h, :],
                                    op0=ALU.mult,
                                    op1=ALU.add,
                                )
                        for qi in range(4):
                            tr = ps_tr.tile([128, 128], BF16, tag="tr")
                            nc.tensor.transpose(
                                tr, oT[:, qi, :], ident[:, :]
                            )
                            nc.vector.tensor_copy(
                                ofin[hp][:, qi * 128:(qi + 1) * 128], tr
                            )
                    # out projection for this q chunk
                    for ts in range(4):
                        tsl = slice(qc * 512 + ts * 128,
                                    qc * 512 + (ts + 1) * 128)
                        for nh in range(2):
                            nsl = slice(nh * 512, (nh + 1) * 512)
                            op = ps_out.tile([128, 512], F32, tag="oproj")
                            nc.tensor.matmul(
                                op,
                                lhsT=ofin[0][:, ts * 128:(ts + 1) * 128],
                                rhs=wo_sb[:, 0, nsl],
                                start=True, stop=False,
                            )
                            nc.tensor.matmul(
                                op,
                                lhsT=ofin[1][:, ts * 128:(ts + 1) * 128],
                                rhs=wo_sb[:, 1, nsl],
                                start=False, stop=True,
                            )
                            ost = ostpool.tile([128, 512], F32, tag="ost")
                            nc.vector.tensor_copy(ost, op)
                            nc.sync.dma_start(out=partial[tsl, nsl], in_=ost)

    nc.finalize()
    return nc


_NC_CACHE = {}


def _get_nc(KT):
    if KT not in _NC_CACHE:
        _NC_CACHE[KT] = _build_nc(KT)
    return _NC_CACHE[KT]


def kernel(q_in, k_in, v_in, kv_pad_mask, Wq, bq, Wk, bk, Wv, bv, Wo, bo,
           _trace=False):
    f32 = np.float32
    bf = ml_dtypes.bfloat16
    q_in = np.asarray(q_in, f32)
    k_in = np.asarray(k_in, f32)
    v_in = np.asarray(v_in, f32)
    mask = np.asarray(kv_pad_mask, bool)
    Wq, bq, Wk, bk, Wv, bv, Wo, bo = (
        np.asarray(a, f32) for a in (Wq, bq, Wk, bk, Wv, bv, Wo, bo)
    )

    # host-side compaction of valid keys, padded to a multiple of 128
    valid_idx = [np.nonzero(~mask[b])[0] for b in range(B)]
    nv = [len(ix) for ix in valid_idx]
    KT = max(1, max((n + 127) // 128 for n in nv))
    LK = KT * 128

    nc = _get_nc(KT)

    fp8 = ml_dtypes.float8_e4m3

    def ilv8(xt):
        # [1024, T] -> DoubleRow-interleaved [128, 4, 2, T] fp8
        T = xt.shape[1]
        return np.ascontiguousarray(
            xt.reshape(NCH // 2, 2, 128, T).transpose(2, 0, 1, 3)
        ).astype(fp8)

    xT = {}
    mb = {}
    for b in range(B):
        ix = valid_idx[b]
        kc = np.zeros((LK, D), f32)
        vc = np.zeros((LK, D), f32)
        kc[:nv[b]] = k_in[b][ix]
        vc[:nv[b]] = v_in[b][ix]
        xT[("q", b)] = ilv8(q_in[b].T)
        xT[("k", b)] = ilv8(kc.T)
        xT[("v", b)] = np.ascontiguousarray(vc.T).astype(bf)
        mrow = np.full(LK, MASK_BIAS, f32)
        mrow[:nv[b]] = 0.0
        mb[b] = np.ascontiguousarray(mrow.reshape(KT, 128).T)

    in_maps = []
    for core in range(N_CORES):
        b = core // 4
        h0 = (core % 4) * HEADS_PER_CORE
        rows = slice(h0 * DH, h0 * DH + HPC)
        nvb = max(float(nv[b]), 1.0)
        # 0.1 * mean_over_valid(v) for this core's 256 dims, replicated
        # across partitions for the stt broadcast operand
        vm = (v_in[b][valid_idx[b]].mean(axis=0) if nv[b] else
              np.zeros(D, f32))
        vm = (vm @ Wv[rows].T + bv[rows]) * EPS_SMOOTH
        vmb = np.broadcast_to(
            vm.reshape(HEADS_PER_CORE, DH), (128, HEADS_PER_CORE, DH)
        )
        in_maps.append({
            "xq8": xT[("q", b)],
            "xk8": xT[("k", b)],
            "xvT": xT[("v", b)],
            "wq8": ilv8(Wq[rows].T * W8SCALE),
            "wk8": ilv8(Wk[rows].T * W8SCALE),
            "wv_t": np.ascontiguousarray(Wv[rows].T).astype(bf),
            "wo_t": np.ascontiguousarray(Wo[:, rows].T).astype(bf),
            "bq": np.ascontiguousarray(
                (bq[rows] * W8SCALE).reshape(2, 1, 128)).astype(bf),
            "bk": np.ascontiguousarray(
                (bk[rows] * W8SCALE).reshape(2, 1, 128)).astype(bf),
            "bv": np.ascontiguousarray(bv[rows].reshape(1, HPC)).astype(bf),
            "mbias": mb[b],
            "vmb": np.ascontiguousarray(vmb.astype(f32)),
        })

    res = run_bass_kernel_spmd(nc, in_maps, core_ids=list(range(N_CORES)),
                               trace=_trace)
    out = np.zeros((B, L, D), f32)
    for core in range(N_CORES):
        out[core // 4] += res.results[core]["partial"]
    out += bo[None, None, :]
    if _trace:
        kernel._last_result = res
    return out


# revision 21
# speedup vs baseline: 1.1062x; 1.1062x over previous
"""Cross-attention kernel for 8 trn2 NeuronCores.

Problem: B=2, Lq=Lk=2048, D=1024, H=16, dh=64.
  q/k/v = Linear(x); q,k L2-normalized per head; S = q@k.T * 1/8;
  key-pad mask -> -1e9; softmax; mask-aware renorm; eps-smooth toward
  uniform-over-valid; out = attn@v merged -> out_proj.

Sharding: core c handles batch b=c//4, heads [4*(c%4), 4*(c%4)+4)
(two head pairs of 2 heads). Each core computes a partial output
projection over its 256 head dims; the host sums the 4 partials per
batch and adds the output bias.

Key optimizations over the v1 baseline:
  - Host-side key compaction: ~half the keys are padding; only valid
    keys (padded to a multiple of 128) are shipped/projected/attended.
    Masked keys contribute exactly 0 to P, rowsum and AV, so dropping
    them is mathematically identical. The Bass program is built per
    runtime KT (cached).
  - Rowsum for free: v carries a 65th column = 1/0.9, so the AV matmul
    output column 64 is rowsum(P)/0.9 and the old per-kt rowsum matmuls
    disappear. AV runs in [q, d] orientation (lhsT = P tile), so the
    renorm divide + eps smoothing collapse into one scalar_tensor_tensor
    with a per-partition 0.9/rs scalar; PE transposes restore [d, q]
    for the output projection.
  - q/k linear bias folded into the Square activation and the final
    normalize stt (no bias matmuls).
  - q-projection pipelined per q-chunk inside the attention loop
    (sharing the S psum tiles), so attention starts after only
    k-proj + v-proj + one q chunk.
  - Out-projection interleaved per q-chunk (no serial tail); the
    0.1*vmean eps-smoothing term rides in the stt; host adds bo.
  - Few large DMAs into resident SBUF x tiles instead of 192 small ones.
"""

import ml_dtypes
import numpy as np

import concourse.bass as bass
from concourse import bacc
from concourse import masks
import concourse.mybir as mybir
import concourse.tile as tile
from concourse.bass_utils import run_bass_kernel_spmd

F32 = mybir.dt.float32
BF16 = mybir.dt.bfloat16
AF = mybir.ActivationFunctionType
ALU = mybir.AluOpType

B, L, D = 2, 2048, 1024
H, DH = 16, 64
HEADS_PER_CORE = 4          # -> 256 dims per core, 2 head pairs
HPC = HEADS_PER_CORE * DH   # 256
SCALE = 0.125               # 1/sqrt(64) / ATTN_TEMP
EPS_SMOOTH = 0.1
INV09 = 1.0 / (1.0 - EPS_SMOOTH)
MASK_BIAS = -30000.0
N_CORES = 8
QC = L // 512               # 4 q chunks
NCH = D // 128              # 8 contraction chunks for projections


def _build_nc(KT):
    LK = KT * 128
    # k-token chunks of <=512 for the k projection
    kchunks = []
    o = 0
    while o < LK:
        sz = min(512, LK - o)
        kchunks.append((o, sz))
        o += sz

    nc = bacc.Bacc(None)

    xqT = nc.dram_tensor("xqT", [D, L], BF16, kind="ExternalInput")
    xkT = nc.dram_tensor("xkT", [D, LK], BF16, kind="ExternalInput")
    xvT = nc.dram_tensor("xvT", [D, LK], BF16, kind="ExternalInput")
    wq_t = nc.dram_tensor("wq_t", [D, HPC], BF16, kind="ExternalInput")
    wk_t = nc.dram_tensor("wk_t", [D, HPC], BF16, kind="ExternalInput")
    wv_t = nc.dram_tensor("wv_t", [D, HPC], BF16, kind="ExternalInput")
    wo_t = nc.dram_tensor("wo_t", [HPC, D], BF16, kind="ExternalInput")
    bq = nc.dram_tensor("bq", [2, 128, 1], F32, kind="ExternalInput")
    bk = nc.dram_tensor("bk", [2, 128, 1], F32, kind="ExternalInput")
    bv = nc.dram_tensor("bv", [1, HPC], BF16, kind="ExternalInput")
    mbias = nc.dram_tensor("mbias", [128, KT], F32, kind="ExternalInput")
    vmb = nc.dram_tensor("vmb", [128, HEADS_PER_CORE, DH], F32,
                         kind="ExternalInput")
    partial = nc.dram_tensor("partial", [L, D], F32, kind="ExternalOutput")

    with tile.TileContext(nc) as tc:
        with (
            tc.tile_pool(name="consts", bufs=1) as consts,
            tc.tile_pool(name="wpool", bufs=1) as wpool,
            tc.tile_pool(name="xres", bufs=1) as xres,
            tc.tile_pool(name="persist", bufs=1) as persist,
            tc.tile_pool(name="l2pool", bufs=4) as l2pool,
            tc.tile_pool(name="ppool", bufs=3) as ppool,
            tc.tile_pool(name="normpool", bufs=4) as normpool,
            tc.tile_pool(name="ostpool", bufs=4) as ostpool,
        ):
            # ---- constants (issued before the big x DMAs) ----
            ones_row = consts.tile([1, 512], BF16, tag="ones_row")
            nc.vector.memset(ones_row, 1.0)
            blockdiag = consts.tile([128, 128], BF16, tag="blockdiag")
            nc.vector.memset(blockdiag, 0.0)
            nc.vector.memset(blockdiag[0:64, 0:64], 1.0)
            nc.vector.memset(blockdiag[64:128, 64:128], 1.0)
            ident = consts.tile([128, 128], BF16, tag="ident")
            masks.make_identity(nc, ident[:, :])
            mbias_sb = consts.tile([128, KT], F32, tag="mbias")
            nc.sync.dma_start(out=mbias_sb, in_=mbias[:, :])
            vmb_sb = consts.tile([128, HEADS_PER_CORE, DH], F32, tag="vmb")
            nc.sync.dma_start(out=vmb_sb, in_=vmb[:, :, :])
            bias_sb = {}
            for name, hnd in (("q", bq), ("k", bk)):
                for hp in range(2):
                    t = consts.tile([128, 1], F32, tag=f"b{name}{hp}")
                    nc.sync.dma_start(out=t, in_=hnd[hp])
                    bias_sb[(name, hp)] = t
            bv_sb = consts.tile([1, HPC], BF16, tag="bv")
            nc.sync.dma_start(out=bv_sb, in_=bv[:, :])

            # ---- weights + x in dependency order (k gates attention) ----
            w_sb = {}
            xq_sb = xres.tile([128, NCH, L], BF16, tag="xq")
            xk_sb = xres.tile([128, NCH, LK], BF16, tag="xk")
            xv_sb = xres.tile([128, NCH, LK], BF16, tag="xv")

            def load_w(name, hnd):
                t = wpool.tile([128, NCH, HPC], BF16, tag=f"w{name}")
                nc.sync.dma_start(
                    out=t, in_=hnd.rearrange("(c p) m -> p c m", p=128)
                )
                w_sb[name] = t

            load_w("k", wk_t)
            for c in range(NCH):
                nc.sync.dma_start(
                    out=xk_sb[:, c, :], in_=xkT[c * 128:(c + 1) * 128, :]
                )
            load_w("v", wv_t)
            for c in range(NCH):
                nc.sync.dma_start(
                    out=xv_sb[:, c, :], in_=xvT[c * 128:(c + 1) * 128, :]
                )
            load_w("q", wq_t)
            for c in range(NCH):
                nc.sync.dma_start(
                    out=xq_sb[:, c, :], in_=xqT[c * 128:(c + 1) * 128, :]
                )
            wo_sb = wpool.tile([128, 2, D], BF16, tag="wo")
            nc.sync.dma_start(
                out=wo_sb, in_=wo_t.rearrange("(h p) m -> p h m", p=128)
            )

            # ---- persistent activations ----
            qTn = [persist.tile([128, L], BF16, tag=f"qTn{hp}", name=f"qTn{hp}")
                   for hp in range(2)]
            kTn = [persist.tile([128, LK], BF16, tag=f"kTn{hp}",
                                name=f"kTn{hp}") for hp in range(2)]
            # v with a 65th column of 1/0.9 per head -> AV col 64 = rs/0.9
            v65 = persist.tile([128, KT, HEADS_PER_CORE, DH + 1], BF16,
                               tag="v65")
            nc.vector.memset(v65[:, :, :, DH:DH + 1], INV09)

            def proj_qk_chunk(pool, name, xsb, dst, t0, tsz):
                """One token chunk of the q/k projection + L2 norm.

                Uses two [128, 1024] psum tiles from `pool` (halves = the
                two head pairs): one for the projection, one for the
                squared-norm blockdiag matmuls. The linear bias rides in
                the Square activation and the final normalize stt.
                """
                pp = pool.tile([128, 1024], F32, tag="s", name=f"pp_{name}")
                n2 = pool.tile([128, 1024], F32, tag="s", name=f"n2_{name}")
                for hp in range(2):
                    hsl = slice(hp * 512, hp * 512 + tsz)
                    for c in range(NCH):
                        nc.tensor.matmul(
                            pp[:, hsl],
                            lhsT=w_sb[name][:, c, hp * 128:(hp + 1) * 128],
                            rhs=xsb[:, c, t0:t0 + tsz],
                            start=(c == 0),
                            stop=(c == NCH - 1),
                        )
                for hp in range(2):
                    hsl = slice(hp * 512, hp * 512 + tsz)
                    bias_col = bias_sb[(name, hp)]
                    # L2 norm over each head's 64 dims of (q + b)
                    sq = l2pool.tile([128, 512], BF16, tag="sq")
                    nc.scalar.activation(
                        sq[:, 0:tsz], pp[:, hsl], AF.Square, bias=bias_col
                    )
                    nc.tensor.matmul(
                        n2[:, hsl], lhsT=blockdiag,
                        rhs=sq[:, 0:tsz], start=True, stop=True,
                    )
                    nlen = l2pool.tile([128, 512], F32, tag="nlen")
                    nc.scalar.activation(nlen[:, 0:tsz], n2[:, hsl], AF.Sqrt)
                    rnorm = l2pool.tile([128, 512], F32, tag="rnorm")
                    nc.vector.reciprocal_approx_fast(
                        rnorm[:, 0:tsz], nlen[:, 0:tsz]
                    )
                    # (q + b) * 1/|q + b|
                    nc.vector.scalar_tensor_tensor(
                        out=dst[hp][:, t0:t0 + tsz],
                        in0=pp[:, hsl],
                        scalar=bias_col,
                        in1=rnorm[:, 0:tsz],
                        op0=ALU.add,
                        op1=ALU.mult,
                    )

            # ---- prologue: k-proj, v-proj, q-proj(chunk 0) ----
            with (
                tc.tile_pool(name="ps_proj", bufs=3, space="PSUM") as ps_proj,
                tc.tile_pool(name="ps_v", bufs=2, space="PSUM") as ps_v,
            ):
                for t0, tsz in kchunks:
                    proj_qk_chunk(ps_proj, "k", xk_sb, kTn, t0, tsz)

                # v: v[t, dout] = sum_c xT[c][:, t].T @ w[c]
                for tt in range(KT):
                    vp = ps_v.tile([128, HPC], F32, tag="vproj")
                    for c in range(NCH):
                        nc.tensor.matmul(
                            vp,
                            lhsT=xv_sb[:, c, tt * 128:(tt + 1) * 128],
                            rhs=w_sb["v"][:, c, :],
                            start=(c == 0), stop=False,
                        )
                    nc.tensor.matmul(
                        vp, lhsT=ones_row[:, 0:128], rhs=bv_sb,
                        start=False, stop=True,
                    )
                    nc.vector.tensor_copy(v65[:, tt, :, 0:DH], vp)

                proj_qk_chunk(ps_proj, "q", xq_sb, qTn, 0, 512)

            # ---- attention + pipelined q-proj + out-projection ----
            with (
                tc.tile_pool(name="ps_S", bufs=2, space="PSUM") as ps_S,
                tc.tile_pool(name="ps_O", bufs=1, space="PSUM") as ps_O,
                tc.tile_pool(name="ps_tr", bufs=1, space="PSUM") as ps_tr,
                tc.tile_pool(name="ps_out", bufs=1, space="PSUM") as ps_out,
                tc.tile_pool(name="ofpool", bufs=2) as ofpool,
            ):
                for qc in range(QC):
                    if qc > 0:
                        # project this chunk's queries (reuses the S psum
                        # tiles; overlaps the previous chunk's attention)
                        proj_qk_chunk(ps_S, "q", xq_sb, qTn, qc * 512, 512)
                    qsl = slice(qc * 512, (qc + 1) * 512)
                    ofin = [
                        ofpool.tile([128, 512], BF16, tag=f"ofin{hp}",
                                    name=f"ofin{hp}")
                        for hp in range(2)
                    ]
                    for hp in range(2):
                        o_ps = [
                            ps_O.tile([128, 4, DH + 1], F32, tag=f"o{h}",
                                      name=f"o{h}")
                            for h in range(2)
                        ]
                        # 8 accumulation regions share these banks at
                        # different free offsets; a start=True reset would
                        # wipe sibling regions, so zero once + accumulate.
                        for h in range(2):
                            nc.vector.memset(o_ps[h], 0.0)
                        for kt in range(KT):
                            s_ps = ps_S.tile([128, 1024], F32, tag="s")
                            ksl = slice(kt * 128, (kt + 1) * 128)
                            # S_T = k̂.T q̂ per head, row-packed (K=64 each)
                            nc.tensor.matmul(
                                s_ps[:, 0:512],
                                lhsT=kTn[hp][0:64, ksl],
                                rhs=qTn[hp][0:64, qsl],
                                start=True, stop=True,
                            )
                            nc.tensor.matmul(
                                s_ps[:, 512:1024],
                                lhsT=kTn[hp][64:128, ksl],
                                rhs=qTn[hp][64:128, qsl],
                                start=True, stop=True,
                            )
                            # P = exp(SCALE*S + pad_bias); padded keys -> 0
                            p_sb = ppool.tile([128, 1024], BF16, tag="p")
                            nc.scalar.activation(
                                p_sb, s_ps, AF.Exp,
                                bias=mbias_sb[:, kt:kt + 1], scale=SCALE,
                            )
                            # O[q, d(+rs)] += P_tile.T @ [v | 1/0.9]
                            for h in range(2):
                                for qi in range(4):
                                    nc.tensor.matmul(
                                        o_ps[h][:, qi, :],
                                        lhsT=p_sb[:, h * 512 + qi * 128:
                                                  h * 512 + (qi + 1) * 128],
                                        rhs=v65[:, kt, hp * 2 + h, :],
                                        start=False, stop=(kt == KT - 1),
                                        skip_group_check=True,
                                    )
                        # renorm + eps smoothing into a head-paired [q, d]
                        # tile, then one [128,128] transpose per q subtile
                        oT = normpool.tile([128, 4, 128], BF16, tag="oT")
                        for h in range(2):
                            rr = normpool.tile([128, 4], F32, tag="rr")
                            nc.vector.reciprocal_approx_fast(
                                rr, o_ps[h][:, :, DH:DH + 1]
                            )
                            for qi in range(4):
                                # (o * 0.9/rs) + 0.1*vmean
                                nc.vector.scalar_tensor_tensor(
                                    out=oT[:, qi, h * DH:(h + 1) * DH],
                                    in0=o_ps[h][:, qi, 0:DH],
                                    scalar=rr[:, qi:qi + 1],
                                    in1=vmb_sb[:, hp * 2 + h, :],
                                    op0=ALU.mult,
                                    op1=ALU.add,
                                )
                        for qi in range(4):
                            tr = ps_tr.tile([128, 128], BF16, tag="tr")
                            nc.tensor.transpose(
                                tr, oT[:, qi, :], ident[:, :]
                            )
                            nc.vector.tensor_copy(
                                ofin[hp][:, qi * 128:(qi + 1) * 128], tr
                            )
                    # out projection for this q chunk
                    for ts in range(4):
                        tsl = slice(qc * 512 + ts * 128,
                                    qc * 512 + (ts + 1) * 128)
                        for nh in range(2):
                            nsl = slice(nh * 512, (nh + 1) * 512)
                            op = ps_out.tile([128, 512], F32, tag="oproj")
                            nc.tensor.matmul(
                                op,
                                lhsT=ofin[0][:, ts * 128:(ts + 1) * 128],
                                rhs=wo_sb[:, 0, nsl],
                                start=True, stop=False,
                            )
                            nc.tensor.matmul(
                                op,
                                lhsT=ofin[1][:, ts * 128:(ts + 1) * 128],
                                rhs=wo_sb[:, 1, nsl],
                                start=False, stop=True,
                            )
                            ost = ostpool.tile([128, 512], F32, tag="ost")
                            nc.vector.tensor_copy(ost, op)
                            nc.sync.dma_start(out=partial[tsl, nsl], in_=ost)

    nc.finalize()
    return nc


_NC_CACHE = {}


def _get_nc(KT):
    if KT not in _NC_CACHE:
        _NC_CACHE[KT] = _build_nc(KT)
    return _NC_CACHE[KT]


def kernel(q_in, k_in, v_in, kv_pad_mask, Wq, bq, Wk, bk, Wv, bv, Wo, bo,
           _trace=False):
    f32 = np.float32
    bf = ml_dtypes.bfloat16
    q_in = np.asarray(q_in, f32)
    k_in = np.asarray(k_in, f32)
    v_in = np.asarray(v_in, f32)
    mask = np.asarray(kv_pad_mask, bool)
    Wq, bq, Wk, bk, Wv, bv, Wo, bo = (
        np.asarray(a, f32) for a in (Wq, bq, Wk, bk, Wv, bv, Wo, bo)
    )

    # host-side compaction of valid keys, padded to a multiple of 128
    valid_idx = [np.nonzero(~mask[b])[0] for b in range(B)]
    nv = [len(ix) for ix in valid_idx]
    KT = max(1, max((n + 127) // 128 for n in nv))
    LK = KT * 128

    nc = _get_nc(KT)

    xT = {}
    mb = {}
    for b in range(B):
        ix = valid_idx[b]
        kc = np.zeros((LK, D), f32)
        vc = np.zeros((LK, D), f32)
        kc[:nv[b]] = k_in[b][ix]
        vc[:nv[b]] = v_in[b][ix]
        xT[("q", b)] = np.ascontiguousarray(q_in[b].T).astype(bf)
        xT[("k", b)] = np.ascontiguousarray(kc.T).astype(bf)
        xT[("v", b)] = np.ascontiguousarray(vc.T).astype(bf)
        mrow = np.full(LK, MASK_BIAS, f32)
        mrow[:nv[b]] = 0.0
        mb[b] = np.ascontiguousarray(mrow.reshape(KT, 128).T)

    in_maps = []
    for core in range(N_CORES):
        b = core // 4
        h0 = (core % 4) * HEADS_PER_CORE
        rows = slice(h0 * DH, h0 * DH + HPC)
        # 0.1 * mean_over_valid(v) for this core's 256 dims, replicated
        # across partitions for the stt broadcast operand
        vm = (v_in[b][valid_idx[b]].mean(axis=0) if nv[b] else
              np.zeros(D, f32))
        vm = (vm @ Wv[rows].T + bv[rows]) * EPS_SMOOTH
        vmb = np.broadcast_to(
            vm.reshape(HEADS_PER_CORE, DH), (128, HEADS_PER_CORE, DH)
        )
        in_maps.append({
            "xqT": xT[("q", b)],
            "xkT": xT[("k", b)],
            "xvT": xT[("v", b)],
            "wq_t": np.ascontiguousarray(Wq[rows].T).astype(bf),
            "wk_t": np.ascontiguousarray(Wk[rows].T).astype(bf),
            "wv_t": np.ascontiguousarray(Wv[rows].T).astype(bf),
            "wo_t": np.ascontiguousarray(Wo[:, rows].T).astype(bf),
            "bq": np.ascontiguousarray(bq[rows].reshape(2, 128, 1)),
            "bk": np.ascontiguousarray(bk[rows].reshape(2, 128, 1)),
            "bv": np.ascontiguousarray(bv[rows].reshape(1, HPC)).astype(bf),
            "mbias": mb[b],
            "vmb": np.ascontiguousarray(vmb.astype(f32)),
        })

    res = run_bass_kernel_spmd(nc, in_maps, core_ids=list(range(N_CORES)),
                               trace=_trace)
    out = np.zeros((B, L, D), f32)
    for core in range(N_CORES):
        out[core // 4] += res.results[core]["partial"]
    out += bo[None, None, :]
    if _trace:
        kernel._last_result = res
    return out


# revision 25
# speedup vs baseline: 1.2264x; 1.1087x over previous
"""Cross-attention kernel for 8 trn2 NeuronCores.

Problem: B=2, Lq=Lk=2048, D=1024, H=16, dh=64.
  q/k/v = Linear(x); q,k L2-normalized per head; S = q@k.T * 1/8;
  key-pad mask -> -1e9; softmax; mask-aware renorm; eps-smooth toward
  uniform-over-valid; out = attn@v merged -> out_proj.

Sharding: core c handles batch b=c//4, heads [4*(c%4), 4*(c%4)+4)
(two head pairs of 2 heads). Each core computes a partial output
projection over its 256 head dims; the host sums the 4 partials per
batch and adds the output bias.

Key optimizations over the v1 baseline:
  - Host-side key compaction: ~half the keys are padding; only valid
    keys (padded to a multiple of 128) are shipped/projected/attended.
    Masked keys contribute exactly 0 to P, rowsum and AV, so dropping
    them is mathematically identical. The Bass program is built per
    runtime KT (cached).
  - Rowsum for free: v carries a 65th column = 1/0.9, so the AV matmul
    output column 64 is rowsum(P)/0.9 and the old per-kt rowsum matmuls
    disappear. AV runs in [q, d] orientation (lhsT = P tile), so the
    renorm divide + eps smoothing collapse into one scalar_tensor_tensor
    with a per-partition 0.9/rs scalar; PE transposes restore [d, q]
    for the output projection.
  - q/k linear bias folded into the Square activation and the final
    normalize stt (no bias matmuls).
  - q-projection pipelined per q-chunk inside the attention loop
    (sharing the S psum tiles), so attention starts after only
    k-proj + v-proj + one q chunk.
  - Out-projection interleaved per q-chunk (no serial tail); the
    0.1*vmean eps-smoothing term rides in the stt; host adds bo.
  - Few large DMAs into resident SBUF x tiles instead of 192 small ones.
"""

import ml_dtypes
import numpy as np

import concourse.bass as bass
from concourse import bacc
from concourse import masks
import concourse.mybir as mybir
import concourse.tile as tile
from concourse.bass_utils import run_bass_kernel_spmd

F32 = mybir.dt.float32
BF16 = mybir.dt.bfloat16
AF = mybir.ActivationFunctionType
ALU = mybir.AluOpType

B, L, D = 2, 2048, 1024
H, DH = 16, 64
HEADS_PER_CORE = 4          # -> 256 dims per core, 2 head pairs
HPC = HEADS_PER_CORE * DH   # 256
SCALE = 0.125               # 1/sqrt(64) / ATTN_TEMP
EPS_SMOOTH = 0.1
INV09 = 1.0 / (1.0 - EPS_SMOOTH)
MASK_BIAS = -30000.0
N_CORES = 8
QC = L // 512               # 4 q chunks
NCH = D // 128              # 8 contraction chunks for projections


def _build_nc(KT):
    LK = KT * 128
    # k-token chunks of <=512 for the k projection
    kchunks = []
    o = 0
    while o < LK:
        sz = min(512, LK - o)
        kchunks.append((o, sz))
        o += sz

    nc = bacc.Bacc(None)

    xqT = nc.dram_tensor("xqT", [D, L], BF16, kind="ExternalInput")
    xkT = nc.dram_tensor("xkT", [D, LK], BF16, kind="ExternalInput")
    xvT = nc.dram_tensor("xvT", [D, LK], BF16, kind="ExternalInput")
    wq_t = nc.dram_tensor("wq_t", [D, HPC], BF16, kind="ExternalInput")
    wk_t = nc.dram_tensor("wk_t", [D, HPC], BF16, kind="ExternalInput")
    wv_t = nc.dram_tensor("wv_t", [D, HPC], BF16, kind="ExternalInput")
    wo_t = nc.dram_tensor("wo_t", [HPC, D], BF16, kind="ExternalInput")
    bq = nc.dram_tensor("bq", [2, 128, 1], F32, kind="ExternalInput")
    bk = nc.dram_tensor("bk", [2, 128, 1], F32, kind="ExternalInput")
    bv = nc.dram_tensor("bv", [1, HPC], BF16, kind="ExternalInput")
    mbias = nc.dram_tensor("mbias", [128, KT], F32, kind="ExternalInput")
    vmb = nc.dram_tensor("vmb", [128, HEADS_PER_CORE, DH], F32,
                         kind="ExternalInput")
    partial = nc.dram_tensor("partial", [L, D], BF16, kind="ExternalOutput")

    with tile.TileContext(nc) as tc:
        with (
            tc.tile_pool(name="consts", bufs=1) as consts,
            tc.tile_pool(name="wpool", bufs=1) as wpool,
            tc.tile_pool(name="xres", bufs=1) as xres,
            tc.tile_pool(name="persist", bufs=1) as persist,
            tc.tile_pool(name="l2pool", bufs=4) as l2pool,
            tc.tile_pool(name="ppool", bufs=3) as ppool,
            tc.tile_pool(name="normpool", bufs=4) as normpool,
            tc.tile_pool(name="ostpool", bufs=4) as ostpool,
        ):
            # ---- constants (issued before the big x DMAs) ----
            ones_row = consts.tile([1, 512], BF16, tag="ones_row")
            nc.vector.memset(ones_row, 1.0)
            blockdiag = consts.tile([128, 128], BF16, tag="blockdiag")
            nc.vector.memset(blockdiag, 0.0)
            nc.vector.memset(blockdiag[0:64, 0:64], 1.0)
            nc.vector.memset(blockdiag[64:128, 64:128], 1.0)
            ident = consts.tile([128, 128], BF16, tag="ident")
            masks.make_identity(nc, ident[:, :])
            mbias_sb = consts.tile([128, KT], F32, tag="mbias")
            nc.sync.dma_start(out=mbias_sb, in_=mbias[:, :])
            vmb_sb = consts.tile([128, HEADS_PER_CORE, DH], F32, tag="vmb")
            nc.sync.dma_start(out=vmb_sb, in_=vmb[:, :, :])
            bias_sb = {}
            for name, hnd in (("q", bq), ("k", bk)):
                for hp in range(2):
                    t = consts.tile([128, 1], F32, tag=f"b{name}{hp}")
                    nc.sync.dma_start(out=t, in_=hnd[hp])
                    bias_sb[(name, hp)] = t
            bv_sb = consts.tile([1, HPC], BF16, tag="bv")
            nc.sync.dma_start(out=bv_sb, in_=bv[:, :])

            # ---- weights + x in dependency order (k gates attention) ----
            w_sb = {}
            xq_sb = xres.tile([128, NCH, L], BF16, tag="xq")
            xk_sb = xres.tile([128, NCH, LK], BF16, tag="xk")
            xv_sb = xres.tile([128, NCH, LK], BF16, tag="xv")

            def load_w(name, hnd):
                t = wpool.tile([128, NCH, HPC], BF16, tag=f"w{name}")
                nc.sync.dma_start(
                    out=t, in_=hnd.rearrange("(c p) m -> p c m", p=128)
                )
                w_sb[name] = t

            load_w("k", wk_t)
            for c in range(NCH):
                nc.sync.dma_start(
                    out=xk_sb[:, c, :], in_=xkT[c * 128:(c + 1) * 128, :]
                )
            load_w("v", wv_t)
            for c in range(NCH):
                nc.sync.dma_start(
                    out=xv_sb[:, c, :], in_=xvT[c * 128:(c + 1) * 128, :]
                )
            load_w("q", wq_t)
            for c in range(NCH):
                nc.sync.dma_start(
                    out=xq_sb[:, c, :], in_=xqT[c * 128:(c + 1) * 128, :]
                )
            wo_sb = wpool.tile([128, 2, D], BF16, tag="wo")
            nc.sync.dma_start(
                out=wo_sb, in_=wo_t.rearrange("(h p) m -> p h m", p=128)
            )

            # ---- persistent activations ----
            qTn = [persist.tile([128, L], BF16, tag=f"qTn{hp}", name=f"qTn{hp}")
                   for hp in range(2)]
            kTn = [persist.tile([128, LK], BF16, tag=f"kTn{hp}",
                                name=f"kTn{hp}") for hp in range(2)]
            # v with a 65th column of 1/0.9 per head -> AV col 64 = rs/0.9
            v65 = persist.tile([128, KT, HEADS_PER_CORE, DH + 1], BF16,
                               tag="v65")
            nc.vector.memset(v65[:, :, :, DH:DH + 1], INV09)

            def proj_qk_chunk(pool, name, xsb, dst, t0, tsz):
                """One token chunk of the q/k projection + L2 norm.

                Uses two [128, 1024] psum tiles from `pool` (halves = the
                two head pairs): one for the projection, one for the
                squared-norm blockdiag matmuls. The linear bias rides in
                the Square activation and the final normalize stt.
                """
                pp = pool.tile([128, 1024], F32, tag="s", name=f"pp_{name}")
                n2 = pool.tile([128, 1024], F32, tag="s", name=f"n2_{name}")
                for hp in range(2):
                    hsl = slice(hp * 512, hp * 512 + tsz)
                    for c in range(NCH):
                        nc.tensor.matmul(
                            pp[:, hsl],
                            lhsT=w_sb[name][:, c, hp * 128:(hp + 1) * 128],
                            rhs=xsb[:, c, t0:t0 + tsz],
                            start=(c == 0),
                            stop=(c == NCH - 1),
                        )
                for hp in range(2):
                    hsl = slice(hp * 512, hp * 512 + tsz)
                    bias_col = bias_sb[(name, hp)]
                    # L2 norm over each head's 64 dims of (q + b)
                    sq = l2pool.tile([128, 512], BF16, tag="sq")
                    nc.scalar.activation(
                        sq[:, 0:tsz], pp[:, hsl], AF.Square, bias=bias_col
                    )
                    nc.tensor.matmul(
                        n2[:, hsl], lhsT=blockdiag,
                        rhs=sq[:, 0:tsz], start=True, stop=True,
                    )
                    nlen = l2pool.tile([128, 512], F32, tag="nlen")
                    nc.scalar.activation(nlen[:, 0:tsz], n2[:, hsl], AF.Sqrt)
                    rnorm = l2pool.tile([128, 512], F32, tag="rnorm")
                    nc.vector.reciprocal_approx_fast(
                        rnorm[:, 0:tsz], nlen[:, 0:tsz]
                    )
                    # (q + b) * 1/|q + b|
                    nc.vector.scalar_tensor_tensor(
                        out=dst[hp][:, t0:t0 + tsz],
                        in0=pp[:, hsl],
                        scalar=bias_col,
                        in1=rnorm[:, 0:tsz],
                        op0=ALU.add,
                        op1=ALU.mult,
                    )

            # ---- prologue: k-proj, v-proj, q-proj(chunk 0) ----
            with (
                tc.tile_pool(name="ps_proj", bufs=3, space="PSUM") as ps_proj,
                tc.tile_pool(name="ps_v", bufs=2, space="PSUM") as ps_v,
            ):
                for t0, tsz in kchunks:
                    proj_qk_chunk(ps_proj, "k", xk_sb, kTn, t0, tsz)

                # v: v[t, dout] = sum_c xT[c][:, t].T @ w[c]
                for tt in range(KT):
                    vp = ps_v.tile([128, HPC], F32, tag="vproj")
                    for c in range(NCH):
                        nc.tensor.matmul(
                            vp,
                            lhsT=xv_sb[:, c, tt * 128:(tt + 1) * 128],
                            rhs=w_sb["v"][:, c, :],
                            start=(c == 0), stop=False,
                        )
                    nc.tensor.matmul(
                        vp, lhsT=ones_row[:, 0:128], rhs=bv_sb,
                        start=False, stop=True,
                    )
                    nc.vector.tensor_copy(v65[:, tt, :, 0:DH], vp)

                for i in range(QC):
                    proj_qk_chunk(ps_proj, "q", xq_sb, qTn, i * 512, 512)

            # ---- attention + pipelined q-proj + out-projection ----
            with (
                tc.tile_pool(name="ps_S", bufs=2, space="PSUM") as ps_S,
                tc.tile_pool(name="ps_O", bufs=1, space="PSUM") as ps_O,
                tc.tile_pool(name="ps_tr", bufs=1, space="PSUM") as ps_tr,
                tc.tile_pool(name="ps_out", bufs=1, space="PSUM") as ps_out,
                tc.tile_pool(name="ofpool", bufs=2) as ofpool,
            ):
                for qc in range(QC):
                    qsl = slice(qc * 512, (qc + 1) * 512)
                    ofin = [
                        ofpool.tile([128, 512], BF16, tag=f"ofin{hp}",
                                    name=f"ofin{hp}")
                        for hp in range(2)
                    ]
                    for hp in range(2):
                        o_ps = [
                            ps_O.tile([128, 4, DH + 1], F32, tag=f"o{h}",
                                      name=f"o{h}")
                            for h in range(2)
                        ]
                        # 8 accumulation regions share these banks at
                        # different free offsets; a start=True reset would
                        # wipe sibling regions, so zero once + accumulate.
                        for h in range(2):
                            nc.vector.memset(o_ps[h], 0.0)
                        for kt in range(KT):
                            s_ps = ps_S.tile([128, 1024], F32, tag="s")
                            ksl = slice(kt * 128, (kt + 1) * 128)
                            # S_T = k̂.T q̂ per head, row-packed (K=64 each)
                            nc.tensor.matmul(
                                s_ps[:, 0:512],
                                lhsT=kTn[hp][0:64, ksl],
                                rhs=qTn[hp][0:64, qsl],
                                start=True, stop=True,
                            )
                            nc.tensor.matmul(
                                s_ps[:, 512:1024],
                                lhsT=kTn[hp][64:128, ksl],
                                rhs=qTn[hp][64:128, qsl],
                                start=True, stop=True,
                            )
                            # P = exp(SCALE*S + pad_bias); padded keys -> 0
                            p_sb = ppool.tile([128, 1024], BF16, tag="p")
                            nc.scalar.activation(
                                p_sb, s_ps, AF.Exp,
                                bias=mbias_sb[:, kt:kt + 1], scale=SCALE,
                            )
                            # O[q, d(+rs)] += P_tile.T @ [v | 1/0.9]
                            for h in range(2):
                                for qi in range(4):
                                    nc.tensor.matmul(
                                        o_ps[h][:, qi, :],
                                        lhsT=p_sb[:, h * 512 + qi * 128:
                                                  h * 512 + (qi + 1) * 128],
                                        rhs=v65[:, kt, hp * 2 + h, :],
                                        start=False, stop=(kt == KT - 1),
                                        skip_group_check=True,
                                    )
                        # renorm + eps smoothing into a head-paired [q, d]
                        # tile, then one [128,128] transpose per q subtile
                        oT = normpool.tile([128, 4, 128], BF16, tag="oT")
                        for h in range(2):
                            rr = normpool.tile([128, 4], F32, tag="rr")
                            nc.vector.reciprocal_approx_fast(
                                rr, o_ps[h][:, :, DH:DH + 1]
                            )
                            for qi in range(4):
                                # (o * 0.9/rs) + 0.1*vmean
                                nc.vector.scalar_tensor_tensor(
                                    out=oT[:, qi, h * DH:(h + 1) * DH],
                                    in0=o_ps[h][:, qi, 0:DH],
                                    scalar=rr[:, qi:qi + 1],
                                    in1=vmb_sb[:, hp * 2 + h, :],
                                    op0=ALU.mult,
                                    op1=ALU.add,
                                )
                        for qi in range(4):
                            tr = ps_tr.tile([128, 128], BF16, tag="tr")
                            nc.tensor.transpose(
                                tr, oT[:, qi, :], ident[:, :]
                            )
                            nc.vector.tensor_copy(
                                ofin[hp][:, qi * 128:(qi + 1) * 128], tr
                            )
                            if hp == 1:
                                # out projection for this token subtile
                                # (both head pairs now final)
                                tsl = slice(qc * 512 + qi * 128,
                                            qc * 512 + (qi + 1) * 128)
                                for nh in range(2):
                                    nsl = slice(nh * 512, (nh + 1) * 512)
                                    op = ps_out.tile([128, 512], F32,
                                                     tag="oproj")
                                    nc.tensor.matmul(
                                        op,
                                        lhsT=ofin[0][:, qi * 128:
                                                     (qi + 1) * 128],
                                        rhs=wo_sb[:, 0, nsl],
                                        start=True, stop=False,
                                    )
                                    nc.tensor.matmul(
                                        op,
                                        lhsT=ofin[1][:, qi * 128:
                                                     (qi + 1) * 128],
                                        rhs=wo_sb[:, 1, nsl],
                                        start=False, stop=True,
                                    )
                                    ost = ostpool.tile([128, 512], BF16,
                                                       tag="ost")
                                    nc.vector.tensor_copy(ost, op)
                                    nc.sync.dma_start(out=partial[tsl, nsl],
                                                      in_=ost)

    nc.finalize()
    return nc


_NC_CACHE = {}


def _get_nc(KT):
    if KT not in _NC_CACHE:
        _NC_CACHE[KT] = _build_nc(KT)
    return _NC_CACHE[KT]


def kernel(q_in, k_in, v_in, kv_pad_mask, Wq, bq, Wk, bk, Wv, bv, Wo, bo,
           _trace=False):
    f32 = np.float32
    bf = ml_dtypes.bfloat16
    q_in = np.asarray(q_in, f32)
    k_in = np.asarray(k_in, f32)
    v_in = np.asarray(v_in, f32)
    mask = np.asarray(kv_pad_mask, bool)
    Wq, bq, Wk, bk, Wv, bv, Wo, bo = (
        np.asarray(a, f32) for a in (Wq, bq, Wk, bk, Wv, bv, Wo, bo)
    )

    # host-side compaction of valid keys, padded to a multiple of 128
    valid_idx = [np.nonzero(~mask[b])[0] for b in range(B)]
    nv = [len(ix) for ix in valid_idx]
    KT = max(1, max((n + 127) // 128 for n in nv))
    LK = KT * 128

    nc = _get_nc(KT)

    xT = {}
    mb = {}
    for b in range(B):
        ix = valid_idx[b]
        kc = np.zeros((LK, D), f32)
        vc = np.zeros((LK, D), f32)
        kc[:nv[b]] = k_in[b][ix]
        vc[:nv[b]] = v_in[b][ix]
        xT[("q", b)] = np.ascontiguousarray(q_in[b].T).astype(bf)
        xT[("k", b)] = np.ascontiguousarray(kc.T).astype(bf)
        xT[("v", b)] = np.ascontiguousarray(vc.T).astype(bf)
        mrow = np.full(LK, MASK_BIAS, f32)
        mrow[:nv[b]] = 0.0
        mb[b] = np.ascontiguousarray(mrow.reshape(KT, 128).T)

    in_maps = []
    for core in range(N_CORES):
        b = core // 4
        h0 = (core % 4) * HEADS_PER_CORE
        rows = slice(h0 * DH, h0 * DH + HPC)
        # 0.1 * mean_over_valid(v) for this core's 256 dims, replicated
        # across partitions for the stt broadcast operand
        vm = (v_in[b][valid_idx[b]].mean(axis=0) if nv[b] else
              np.zeros(D, f32))
        vm = (vm @ Wv[rows].T + bv[rows]) * EPS_SMOOTH
        vmb = np.broadcast_to(
            vm.reshape(HEADS_PER_CORE, DH), (128, HEADS_PER_CORE, DH)
        )
        in_maps.append({
            "xqT": xT[("q", b)],
            "xkT": xT[("k", b)],
            "xvT": xT[("v", b)],
            "wq_t": np.ascontiguousarray(Wq[rows].T).astype(bf),
            "wk_t": np.ascontiguousarray(Wk[rows].T).astype(bf),
            "wv_t": np.ascontiguousarray(Wv[rows].T).astype(bf),
            "wo_t": np.ascontiguousarray(Wo[:, rows].T).astype(bf),
            "bq": np.ascontiguousarray(bq[rows].reshape(2, 128, 1)),
            "bk": np.ascontiguousarray(bk[rows].reshape(2, 128, 1)),
            "bv": np.ascontiguousarray(bv[rows].reshape(1, HPC)).astype(bf),
            "mbias": mb[b],
            "vmb": np.ascontiguousarray(vmb.astype(f32)),
        })

    res = run_bass_kernel_spmd(nc, in_maps, core_ids=list(range(N_CORES)),
                               trace=_trace)
    out = np.zeros((B, L, D), f32)
    for core in range(N_CORES):
        out[core // 4] += res.results[core]["partial"]
    out += bo[None, None, :]
    if _trace:
        kernel._last_result = res
    return out
